# revision 24
# baseline (speedup 1.0000x reference)
"""Trainium2 Bass kernel for a PointNet-style neighborhood encoder.

Computation (matches the reference nn.Module):
    h = relu(relu(relu(points @ W0 + b0) @ W1 + b1) @ W2 + b2)   # [N,3] -> [N,128]
    pooled = segment_max(h, cluster)                             # [C,128], 32 pts/cluster
    out = relu(relu(pooled @ G0 + g0) @ G1 + g1)                 # [C,256]

Sharding: data-parallel over points across 8 NeuronCores (cluster
boundaries are shard-aligned because clusters are contiguous, 32
points each). Weights are replicated. No collectives; the host
scatters inputs and gathers per-core outputs.

Device strategy (per core, n = 262144 points = 65536 quad-columns):
  - Host packs points feature-major, 4 points per 128-partition column
    ("quads"): pts4[3a+f, q] = points[4q+a, f], so layer 0 is a single
    block-diagonal matmul (K=12, M=128) producing h0 for 4 points/col.
  - Layer 1 uses two permuted block-diagonal stationaries W1A/W1B
    (K=128, M=128) producing h1 with 2 points per column.
  - Layer 2 uses W2 duplicated on both partition halves; 4 sub-matmuls
    (K=64, M=128) with rhs partition slices map to distinct PE row
    groups, producing z = W2^T h1 (bias/relu deferred) in PSUM.
  - segment_max: relu is monotone and b2 is constant per feature, so
    pooled = relu(max_p(z) + b2). max over (4 tensors x 8 quads) is ONE
    VectorE tensor_reduce(axis=XY) straight out of PSUM per sub-chunk.
  - ScalarE (ACT) does every relu+bias PSUM->SBUF evacuation; VectorE
    only does the pooling reduces. bf16 activations everywhere
    (PSUM stays f32 as the HW requires).
  - Global MLP on pooled [128, 8192] per core; output is written
    feature-major [256, 8192] bf16 and transposed/upcast on the host.
"""

import numpy as np

# ---- problem geometry (hardcoded per contract) ----
N = 2097152          # total points
C = 65536            # clusters
PTS = 32             # points per cluster
NCORES = 8
NPC = N // NCORES    # points per core = 262144
N4C = NPC // 4       # quad-columns per core = 65536
CPC = C // NCORES    # clusters per core = 8192

BIG = 1024           # quad-columns per big-chunk
SUB = 256            # quad-columns per L2/pool sub-chunk
NCHUNK = N4C // BIG  # 64
NSUB = BIG // SUB    # 4

_CACHE = {}


def _bf16():
    import ml_dtypes
    return ml_dtypes.bfloat16


def _build_module(n4c: int):
    """Build the Bass module (SPMD program, same for all cores)."""
    import concourse.bass as bass
    import concourse.bacc as bacc
    import concourse.tile as tile
    from concourse import mybir

    BF = mybir.dt.bfloat16
    F32 = mybir.dt.float32
    RELU = mybir.ActivationFunctionType.Relu
    MAX = mybir.AluOpType.max
    ADD = mybir.AluOpType.add
    XY = mybir.AxisListType.XY

    nchunk = n4c // BIG
    cpc = n4c // 8          # clusters per core for this size

    nc = bacc.Bacc()

    # ---- DRAM I/O ----
    # weights batched into two tensors so the prologue is 3 parallel DMAs
    # (13 small serialized DMAs used to block the Pool queue for ~6.5us)
    pts4 = nc.dram_tensor("pts4", [12, n4c], BF, kind="ExternalInput")
    w0q = nc.dram_tensor("w0q", [12, 128], BF, kind="ExternalInput")
    wbig = nc.dram_tensor("wbig", [128, 768], BF, kind="ExternalInput")
    bbig = nc.dram_tensor("bbig", [128, 6], F32, kind="ExternalInput")
    outt = nc.dram_tensor("outt", [256, cpc], BF, kind="ExternalOutput")

    from contextlib import ExitStack
    with tile.TileContext(nc) as tc, ExitStack() as ctx:
        singles = ctx.enter_context(tc.tile_pool(name="singles", bufs=1))
        ppts = ctx.enter_context(tc.tile_pool(name="ppts", bufs=3))
        ph0s = ctx.enter_context(tc.tile_pool(name="ph0s", bufs=2))
        ph1s = ctx.enter_context(tc.tile_pool(name="ph1s", bufs=3))
        psT = ctx.enter_context(tc.tile_pool(name="psT", bufs=2))
        psum_h1 = ctx.enter_context(tc.tile_pool(name="psum_h1", bufs=2, space="PSUM"))
        psum_z = ctx.enter_context(tc.tile_pool(name="psum_z", bufs=2, space="PSUM"))

        # ---- load constants: 3 batched DMAs on otherwise-idle queues ----
        w0q_s = singles.tile([12, 128], BF)
        nc.gpsimd.dma_start(out=w0q_s[:], in_=w0q[:])
        wbig_s = singles.tile([128, 768], BF)
        nc.gpsimd.dma_start(out=wbig_s[:], in_=wbig[:])
        bbig_s = singles.tile([128, 6], F32)
        nc.scalar.dma_start(out=bbig_s[:], in_=bbig[:])
        w1a_s = wbig_s[:, 0:128]
        w1b_s = wbig_s[:, 128:256]
        w2d_s = wbig_s[:, 256:384]
        g0w_s = wbig_s[:, 384:512]
        g1lo_s = wbig_s[:, 512:640]
        g1hi_s = wbig_s[:, 640:768]
        b0q_s = bbig_s[:, 0:1]
        b1d_s = bbig_s[:, 1:2]
        b2v_s = bbig_s[:, 2:3]
        g0v_s = bbig_s[:, 3:4]
        g1l_s = bbig_s[:, 4:5]
        g1h_s = bbig_s[:, 5:6]

        # pooled max(z) accumulator for the whole core
        pooled = singles.tile([128, cpc], BF)

        # ---- main loop ----
        # Real-HW constraints shape everything here: only ACT (unary) and
        # DVE (at most one PSUM operand per instruction) may read PSUM; the
        # Pool engine is SBUF-only; DMA cannot touch PSUM.
        # Per chunk (1024 quad-cols = 4096 points) steady-state budget:
        #   DVE  3x tensor_reduce (one [128,1024] z sub -> 32 pooled cols,
        #        input-bound) + 1x TT merge for the 4th sub      ~4230 ns
        #   ACT  h0 evac [1024] + 2x h1 evac [1024] + sub-3 half
        #        evac [512] + G-MLP psum evacs                   ~4190 ns
        #   PE   L0 2x512 + L1 4x512 + L2 16x256 + G mms         ~3150 ns
        #   Pool sub-3 SBUF max tree + g0in                       ~550 ns
        # PSUM (8 banks): ring A 2x[128,1024] (h0 tile, h1 a/b tiles and
        # the [128,512] G-MLP matmul tiles), ring Z 2x[128,1024] (z subs).
        hist = {}      # chunk -> (h1as, h1bs) awaiting their L2 stage
        h0_cur = None  # h0s of the chunk whose L1 stage runs next

        def emit_sub_reduce(i, j, h1as, h1bs):
            """L2 sub j: 4 matmuls into one [128,1024] z tile laid out
            (t, c, q) with t = 4 point-groups, then one DVE tensor_reduce
            over (t, q) = all 32 points per cluster."""
            s0 = j * SUB
            zp = psum_z.tile([128, 4 * SUB], F32, tag="zp")
            nc.tensor.matmul(zp[:, 0:SUB],
                             w2d_s[0:64, :], h1as[0:64, s0:s0 + SUB])
            nc.tensor.matmul(zp[:, SUB:2 * SUB],
                             w2d_s[0:64, :], h1bs[0:64, s0:s0 + SUB])
            nc.tensor.matmul(zp[:, 2 * SUB:3 * SUB],
                             w2d_s[64:128, :], h1as[64:128, s0:s0 + SUB])
            nc.tensor.matmul(zp[:, 3 * SUB:4 * SUB],
                             w2d_s[64:128, :], h1bs[64:128, s0:s0 + SUB])
            zv = zp.rearrange("p (t c q) -> p c t q", t=4, q=8)
            base = i * (BIG // 8) + j * (SUB // 8)
            nc.vector.tensor_reduce(
                pooled[:, base:base + SUB // 8], zv, axis=XY, op=MAX)

        def emit_sub_tree(i, j, h1as, h1bs):
            """L2 sub j via the split path: ACT copies the (a0,b0) half to
            SBUF (Pool folds it there), DVE reduces the (a1,b1) half from
            PSUM over (t', q), and Pool merges the two 32-col results.
            No saturated-engine cross-dependency: DVE only needs the
            matmuls, the ACT->Pool chain rides Pool's slack."""
            s0 = j * SUB
            zp = psum_z.tile([128, 4 * SUB], F32, tag="zp")
            nc.tensor.matmul(zp[:, 0:SUB],
                             w2d_s[0:64, :], h1as[0:64, s0:s0 + SUB])
            nc.tensor.matmul(zp[:, SUB:2 * SUB],
                             w2d_s[0:64, :], h1bs[0:64, s0:s0 + SUB])
            nc.tensor.matmul(zp[:, 2 * SUB:3 * SUB],
                             w2d_s[64:128, :], h1as[64:128, s0:s0 + SUB])
            nc.tensor.matmul(zp[:, 3 * SUB:4 * SUB],
                             w2d_s[64:128, :], h1bs[64:128, s0:s0 + SUB])
            base = i * (BIG // 8) + j * (SUB // 8)
            # DVE: reduce the PSUM (a1,b1) half over (t', q)
            r3b = psT.tile([128, 32], BF, tag="r3b")
            zv = zp.rearrange("p (t c q) -> p c t q", t=4, q=8)
            nc.vector.tensor_reduce(r3b[:], zv[:, :, 2:4, :], axis=XY,
                                    op=MAX)
            # ACT: copy the (a0,b0) half to SBUF; Pool folds it
            zA = psT.tile([128, 512], BF, tag="zA")
            nc.scalar.copy(out=zA[:], in_=zp[:, 0:512])
            f1 = psT.tile([128, 256], BF, tag="f1")
            nc.gpsimd.tensor_tensor(out=f1[:], in0=zA[:, 0:256],
                                    in1=zA[:, 256:512], op=MAX)
            f1v = f1.rearrange("p (c q) -> p c q", q=8)
            f2 = psT.tile([128, 128], BF, tag="f2")
            nc.gpsimd.tensor_tensor(out=f2[:], in0=f1v[:, :, 0:4],
                                    in1=f1v[:, :, 4:8], op=MAX)
            f2v = f2.rearrange("p (c q) -> p c q", q=4)
            f3 = psT.tile([128, 64], BF, tag="f3")
            nc.gpsimd.tensor_tensor(out=f3[:], in0=f2v[:, :, 0:2],
                                    in1=f2v[:, :, 2:4], op=MAX)
            f3v = f3.rearrange("p (c q) -> p c q", q=2)
            f4 = psT.tile([128, 32], BF, tag="f4")
            nc.gpsimd.tensor_tensor(out=f4[:], in0=f3v[:, :, 0:1],
                                    in1=f3v[:, :, 1:2], op=MAX)
            nc.gpsimd.tensor_tensor(out=pooled[:, base:base + SUB // 8],
                                    in0=f4[:], in1=r3b[:], op=MAX)

        g0in = singles.tile([128, cpc], BF)
        g1in = singles.tile([128, cpc], BF)
        goutL = singles.tile([128, cpc], BF)
        goutH = singles.tile([128, cpc], BF)

        # global-MLP tasks at 512-cluster granularity; the [128,512] matmul
        # tiles ride the psum_h1 ring, psum evacs on ACT, g0in on Pool
        def g_task_g0(k):
            sl = slice(k * 512, (k + 1) * 512)
            nc.gpsimd.tensor_scalar(out=g0in[:, sl], in0=pooled[:, sl],
                                    scalar1=b2v_s[:], scalar2=0.0,
                                    op0=ADD, op1=MAX)
            gp = psum_h1.tile([128, 512], F32, tag="h1p")
            nc.tensor.matmul(gp[:], g0w_s[:], g0in[:, sl])
            nc.scalar.activation(g1in[:, sl], gp[:], RELU, bias=g0v_s[:])

        def g_task_lo(k):
            sl = slice(k * 512, (k + 1) * 512)
            gpl = psum_h1.tile([128, 512], F32, tag="h1p")
            nc.tensor.matmul(gpl[:], g1lo_s[:], g1in[:, sl])
            nc.scalar.activation(goutL[:, sl], gpl[:], RELU, bias=g1l_s[:])
            nc.sync.dma_start(out=outt[0:128, sl], in_=goutL[:, sl])

        def g_task_hi(k):
            sl = slice(k * 512, (k + 1) * 512)
            gph = psum_h1.tile([128, 512], F32, tag="h1p")
            nc.tensor.matmul(gph[:], g1hi_s[:], g1in[:, sl])
            nc.scalar.activation(goutH[:, sl], gph[:], RELU, bias=g1h_s[:])
            nc.sync.dma_start(out=outt[128:256, sl], in_=goutH[:, sl])

        g_tasks = []

        def pop_g_task():
            if g_tasks:
                fn, k = g_tasks.pop(0)
                fn(k)

        for it in range(nchunk + 1):
            do_l0 = it < nchunk
            ip = it - 2          # chunk whose L2+pool stage runs now
            prev_ready = ip >= 0 and ip in hist
            if prev_ready:
                pm = hist[ip]

            if do_l0:
                # L0 of chunk `it`: one [128,1024] PSUM tile on the h1
                # ring, one ACT evacuation
                c0 = it * BIG
                pts_t = ppts.tile([12, BIG], BF, tag="pts")
                nc.sync.dma_start(out=pts_t[:], in_=pts4[:, c0:c0 + BIG])
                h0s_n = ph0s.tile([128, BIG], BF, tag="h0s")
                h0p = psum_h1.tile([128, BIG], F32, tag="h1p")
                nc.tensor.matmul(h0p[:, 0:512], w0q_s[:], pts_t[:, 0:512])
                nc.tensor.matmul(h0p[:, 512:1024], w0q_s[:],
                                 pts_t[:, 512:1024])
                nc.scalar.activation(h0s_n[:], h0p[:], RELU, bias=b0q_s[:])

            if prev_ready:
                emit_sub_reduce(ip, 0, pm[0], pm[1])

            if it == 0:
                h0_cur = h0s_n
                continue
            h0s = h0_cur

            # L1A of chunk it-1
            h1as = ph1s.tile([128, BIG], BF, tag="h1as")
            h1bs = ph1s.tile([128, BIG], BF, tag="h1bs")
            h1pa = psum_h1.tile([128, BIG], F32, tag="h1p")
            nc.tensor.matmul(h1pa[:, 0:512], w1a_s[:], h0s[:, 0:512])
            nc.tensor.matmul(h1pa[:, 512:1024], w1a_s[:], h0s[:, 512:1024])
            nc.scalar.activation(h1as[:], h1pa[:], RELU, bias=b1d_s[:])

            if prev_ready:
                emit_sub_reduce(ip, 1, pm[0], pm[1])

            # L1B of chunk it-1
            h1pb = psum_h1.tile([128, BIG], F32, tag="h1p")
            nc.tensor.matmul(h1pb[:, 0:512], w1b_s[:], h0s[:, 0:512])
            nc.tensor.matmul(h1pb[:, 512:1024], w1b_s[:], h0s[:, 512:1024])
            nc.scalar.activation(h1bs[:], h1pb[:], RELU, bias=b1d_s[:])

            if prev_ready:
                emit_sub_reduce(ip, 2, pm[0], pm[1])
                emit_sub_tree(ip, 3, pm[0], pm[1])
                del hist[ip]
            pop_g_task()

            hist[it - 1] = (h1as, h1bs)
            h0_cur = h0s_n if do_l0 else None

            # global-MLP: chunk c pools during iteration c+2, so block k
            # (chunks 4k..4k+3) is ready from iteration 4k+5 on.
            if it >= 6 and (it - 6) % 4 == 0:
                k = (it - 6) // 4
                g_tasks.extend([(g_task_g0, k), (g_task_lo, k),
                                (g_task_hi, k)])

        # epilogue: the last chunk's L2+pool, then leftover G tasks
        L = nchunk - 1
        pm = hist[L]
        emit_sub_reduce(L, 0, pm[0], pm[1])
        pop_g_task()
        emit_sub_reduce(L, 1, pm[0], pm[1])
        pop_g_task()
        emit_sub_reduce(L, 2, pm[0], pm[1])
        pop_g_task()
        emit_sub_tree(L, 3, pm[0], pm[1])
        del hist[L]
        for fn, k in g_tasks:
            fn(k)
        g_tasks.clear()
        first_unpushed = ((nchunk - 6) // 4 + 1) if nchunk >= 6 else 0
        last_k = cpc // 512 - 1
        for k in range(first_unpushed, last_k + 1):
            g_task_g0(k); g_task_lo(k); g_task_hi(k)

    nc.compile()
    return nc


def _host_pack(points, W0, b0, W1, b1, W2, b2, G0, g0, G1, g1, n4c):
    """Build per-core input maps (host-side layout prep, numpy only)."""
    bf16 = _bf16()
    n = n4c * 4 * NCORES

    # pts4[3a+f, q] = points[4q+a, f]
    pts4 = np.ascontiguousarray(
        points[:n].reshape(-1, 4, 3).transpose(1, 2, 0).reshape(12, -1)
    ).astype(bf16)

    # W0 block-diagonal over 4 points: [12, 128]
    w0q = np.zeros((12, 128), np.float32)
    for a in range(4):
        w0q[3 * a:3 * a + 3, 32 * a:32 * a + 32] = W0
    # W1A/W1B: rows 32a+f; cols 64a'+g ; a' in {0,1} / {2,3}
    w1a = np.zeros((128, 128), np.float32)
    w1b = np.zeros((128, 128), np.float32)
    for a in range(2):
        w1a[32 * a:32 * a + 32, 64 * a:64 * a + 64] = W1
        w1b[32 * (a + 2):32 * (a + 2) + 32, 64 * a:64 * a + 64] = W1
    # W2 duplicated on both partition halves
    w2d = np.concatenate([W2, W2], axis=0)

    wbig = np.concatenate(
        [w1a, w1b, w2d, G0, G1[:, :128], G1[:, 128:]], axis=1)
    bbig = np.stack([np.tile(b0, 4), np.tile(b1, 2), b2, g0,
                     g1[:128], g1[128:]], axis=1)
    common = {
        "w0q": w0q.astype(bf16),
        "wbig": wbig.astype(bf16),
        "bbig": bbig.astype(np.float32),
    }
    in_maps = []
    for c in range(NCORES):
        m = dict(common)
        m["pts4"] = np.ascontiguousarray(pts4[:, c * n4c:(c + 1) * n4c])
        in_maps.append(m)
    return in_maps


def _numpy_fallback(points, cluster, num_clusters,
                    W0, b0, W1, b1, W2, b2, G0, g0, G1, g1):
    h = points.astype(np.float32)
    for W, b in ((W0, b0), (W1, b1), (W2, b2)):
        h = np.maximum(h @ W + b, 0.0)
    order = np.argsort(cluster, kind="stable")
    cs = cluster[order]
    hs = h[order]
    starts = np.searchsorted(cs, np.arange(num_clusters), side="left")
    counts = np.bincount(cs, minlength=num_clusters)
    safe_starts = np.minimum(starts, max(len(hs) - 1, 0))
    seg = np.maximum.reduceat(hs, safe_starts, axis=0)
    seg[counts == 0] = -np.inf   # match segment_max identity on empties
    pooled = seg
    gx = pooled
    for W, b in ((G0, g0), (G1, g1)):
        gx = np.maximum(gx @ W + b, 0.0)
    return gx.astype(np.float32)


def kernel(**inputs) -> np.ndarray:
    points = np.asarray(inputs["points"], np.float32)
    cluster = np.asarray(inputs["cluster"]).astype(np.int64)
    num_clusters = int(np.asarray(inputs["num_clusters"]))
    W0 = np.asarray(inputs["W0"], np.float32); b0 = np.asarray(inputs["b0"], np.float32)
    W1 = np.asarray(inputs["W1"], np.float32); b1 = np.asarray(inputs["b1"], np.float32)
    W2 = np.asarray(inputs["W2"], np.float32); b2 = np.asarray(inputs["b2"], np.float32)
    G0 = np.asarray(inputs["G0"], np.float32); g0 = np.asarray(inputs["g0"], np.float32)
    G1 = np.asarray(inputs["G1"], np.float32); g1 = np.asarray(inputs["g1"], np.float32)

    expected = (points.shape == (N, 3) and num_clusters == C
                and cluster.shape == (N,))
    if expected:
        # contiguous equal clusters of 32 points, as produced by setup_inputs
        expected = bool(
            np.array_equal(cluster[::PTS], np.arange(C, dtype=np.int64))
            and np.array_equal(cluster, np.repeat(cluster[::PTS], PTS))
        )
    if not expected:
        return _numpy_fallback(points, cluster, num_clusters,
                               W0, b0, W1, b1, W2, b2, G0, g0, G1, g1)

    from concourse.bass_utils import run_bass_kernel_spmd

    if "nc" not in _CACHE:
        _CACHE["nc"] = _build_module(N4C)
    nc = _CACHE["nc"]

    in_maps = _host_pack(points, W0, b0, W1, b1, W2, b2, G0, g0, G1, g1, N4C)
    res = run_bass_kernel_spmd(nc, in_maps, core_ids=list(range(NCORES)))
    outs = []
    for c in range(NCORES):
        o = np.asarray(res.results[c]["outt"]).astype(np.float32)  # [256, CPC]
        outs.append(o.T)                                           # [CPC, 256]
    return np.ascontiguousarray(np.concatenate(outs, axis=0))

